# revision 2
# baseline (speedup 1.0000x reference)
# RBF Gram matrix kernel for Trainium2 (8 NeuronCores, SPMD).
#
# reference:  G[i, j] = exp(-gamma * ||x_i - y_j||^2)
#                    = exp(2*gamma*(x@y^T)[i,j] - gamma*||x_i||^2 - gamma*||y_j||^2)
#
# Sharding: row-shard x across 8 cores (1024 rows each), replicate y.
# Each core computes a [1024, 8192] slice of G:
#   PE   : xy = x_c @ y^T   (bf16 inputs, fp32 PSUM, K=512 as 4 k-tiles)
#   DVE  : s  = xy + (-0.5*||y||^2)   (broadcast row, free-dim vector)
#   ACT  : out = Exp(2*gamma*s + (-gamma*||x||^2))  (per-partition bias)
#   DMA  : out tile -> DRAM
import os

import numpy as np
import ml_dtypes

N_CORES = 8
N_FULL = 8192          # rows of x (and of G)
M_FULL = 8192          # rows of y (cols of G)
D = 512                # feature dim (contraction)
MC = N_FULL // N_CORES # 1024 rows of x per core
P = 128                # SBUF partitions
NT = 512               # moving-dim tile (max for fp32 psum bank)
KT = D // P            # 4 k-tiles
MT = MC // P           # 8 m-tiles per core
NTILES = M_FULL // NT  # 16 n-tiles

_cache = {}


def _build_program(scale2g: float, mc: int, n_full: int, d: int):
    """Build + compile the SPMD bass program. scale2g = 2*gamma immediate."""
    import concourse.mybir as mybir
    import concourse.tile as tile
    from concourse import bacc

    mt = mc // P
    kt = d // P
    ntiles = n_full // NT

    nc = bacc.Bacc("TRN2", target_bir_lowering=False, debug=False,
                   num_devices=N_CORES)

    xT_d = nc.dram_tensor("xT", [d, mc], mybir.dt.bfloat16,
                          kind="ExternalInput").ap()
    yT_d = nc.dram_tensor("yT", [d, n_full], mybir.dt.bfloat16,
                          kind="ExternalInput").ap()
    y2_d = nc.dram_tensor("y2n", [P, n_full], mybir.dt.float32,
                          kind="ExternalInput").ap()
    x2_d = nc.dram_tensor("x2b", [P, mt], mybir.dt.float32,
                          kind="ExternalInput").ap()
    out_d = nc.dram_tensor("out", [mc, n_full], mybir.dt.float32,
                           kind="ExternalOutput").ap()

    with tile.TileContext(nc) as tc:
        with (
            tc.tile_pool(name="resident", bufs=1) as res_pool,
            tc.tile_pool(name="psum", bufs=8, space="PSUM") as psum_pool,
            tc.tile_pool(name="sq", bufs=4) as s_pool,
            tc.tile_pool(name="ot", bufs=4) as o_pool,
        ):
            # resident operands
            xT_sb = []
            yT_sb = []
            for k in range(kt):
                t = res_pool.tile([P, mc], mybir.dt.bfloat16, tag=f"xT{k}")
                nc.sync.dma_start(out=t, in_=xT_d[k * P:(k + 1) * P, :])
                xT_sb.append(t)
            for k in range(kt):
                t = res_pool.tile([P, n_full], mybir.dt.bfloat16, tag=f"yT{k}")
                yT_sb.append(t)
            y2_sb = res_pool.tile([P, n_full], mybir.dt.float32, tag="y2")
            x2_sb = res_pool.tile([P, mt], mybir.dt.float32, tag="x2")
            nc.sync.dma_start(out=x2_sb, in_=x2_d)

            # Chunk the big per-k DMAs along n so early n-tiles can start
            # before the whole of y^T has landed.
            NCH = min(2048, n_full)
            for ch in range(n_full // NCH):
                sl = slice(ch * NCH, (ch + 1) * NCH)
                for k in range(kt):
                    nc.sync.dma_start(out=yT_sb[k][:, sl], in_=yT_d[k * P:(k + 1) * P, sl])
                nc.sync.dma_start(out=y2_sb[:, sl], in_=y2_d[:, sl])

            # n outer so compute starts after the first y^T chunk
            for n in range(ntiles):
                nsl = slice(n * NT, (n + 1) * NT)
                for m in range(mt):
                    msl = slice(m * P, (m + 1) * P)
                    ps = psum_pool.tile([P, NT], mybir.dt.float32)
                    for k in range(kt):
                        nc.tensor.matmul(
                            ps,
                            lhsT=xT_sb[k][:, msl],
                            rhs=yT_sb[k][:, nsl],
                            start=(k == 0),
                            stop=(k == kt - 1),
                        )
                    s = s_pool.tile([P, NT], mybir.dt.float32)
                    nc.vector.tensor_add(s, ps, y2_sb[:, nsl])
                    o = o_pool.tile([P, NT], mybir.dt.float32)
                    nc.scalar.activation(
                        o, s, mybir.ActivationFunctionType.Exp,
                        bias=x2_sb[:, m:m + 1], scale=float(scale2g),
                    )
                    nc.sync.dma_start(out=out_d[msl, nsl], in_=o)

    nc.compile()
    return nc


def kernel(x: np.ndarray, y: np.ndarray, gamma: np.ndarray) -> np.ndarray:
    from concourse.bass_utils import run_bass_kernel_spmd

    x = np.asarray(x, dtype=np.float32)
    y = np.asarray(y, dtype=np.float32)
    g = float(np.asarray(gamma))

    n, d = x.shape
    m = y.shape[0]
    assert (n, d, m) == (N_FULL, D, M_FULL), (n, d, m)

    key = (g, n, d, m)
    if key not in _cache:
        _cache.clear()
        _cache[key] = _build_program(2.0 * g, MC, M_FULL, D)
    nc = _cache[key]

    # host-side prep (O(N*D), ~0.01% of kernel FLOPs)
    bf16 = ml_dtypes.bfloat16
    x_b = x.astype(bf16)
    yT = np.ascontiguousarray(y.astype(bf16).T)                     # [D, M]
    y2 = np.einsum("md,md->m", y, y, dtype=np.float64)
    y2n = np.ascontiguousarray(
        np.broadcast_to((-0.5 * y2).astype(np.float32), (P, m)))    # [128, M]
    x2 = np.einsum("nd,nd->n", x, x, dtype=np.float64)

    in_maps = []
    for c in range(N_CORES):
        sl = slice(c * MC, (c + 1) * MC)
        xT_c = np.ascontiguousarray(x_b[sl].T)                      # [D, MC]
        x2_c = np.ascontiguousarray(
            (-g * x2[sl]).astype(np.float32).reshape(MT, P).T)      # [128, MT]
        in_maps.append({"xT": xT_c, "yT": yT, "y2n": y2n, "x2b": x2_c})

    trace = bool(int(os.environ.get("RBF_TRACE", "0")))
    res = run_bass_kernel_spmd(nc, in_maps, core_ids=list(range(N_CORES)),
                               trace=trace)
    global LAST_RESULTS
    LAST_RESULTS = res
    return np.concatenate([r["out"] for r in res.results], axis=0)


LAST_RESULTS = None
